# revision 22
# baseline (speedup 1.0000x reference)
"""BitLinear (BitNet b1.58) forward kernel for Trainium2, 8 NeuronCores.

y = act_quant(x) @ weight_quant(W)^T + bias
  - activation quant: per-token absmax int8 fake-quant (values in [-127,127])
  - weight quant: per-tensor mean-absmax ternary fake-quant {-1,0,1}

Sharding: data-parallel over the batch dim (8 batches -> 1 per core);
W and bias are replicated per core, each core computes mean(|W|) locally
(no collectives needed).

Design notes (tolerance is rel_err < 2e-2 vs output absmax):
  * The matmul runs in bf16: the PE streams its moving operand at
    2 B/lane/cycle no matter the dtype (fp8 DoubleRow packs 2 k-tiles in
    the same bytes = 2x flops, but exact int8 activations then need hi+lo
    fp8 pairs = the same 2 bytes -> parity with bf16; single-fp8
    activations lose ~2.4% > tolerance). q int8 in bf16 (exact), ternary
    weights in bf16 (exact), fp32 PSUM accumulation (exact).
  * Activation round via output-convert RNE: fp16(x*sx127 + 1536) ==
    1536 + round(x*sx127) exactly (fp16 ulp is 1.0 on [1024,2048)); the
    rare double-round tie flips one token-element by 1 quantum (~2e-4),
    harmless. Weights use the single-rounding fp32 +1.5*2^23 magic --
    a weight flip corrupts a whole output column (~3e-2), not harmless.
  * y is stored to HBM as bf16 (halves store traffic) and upcast to fp32
    on the host after the gather. Bias is one fused 2x-rate DVE add on
    the bf16 tile -- the PE does matmuls and transposes only.
  * Startup: W rides the sync ring alone (full bandwidth, ~13us), while
    the first 5 x tiles ride the scalar ring and keep DVE/ACT/PE warm;
    tile heads are stats-independent (c_tok is computed at body time) so
    nothing in a head waits for mean(|W|).

Engine layout per token tile: ACT does the x-scale+round pass and the
two epilogue passes (PSUM * c_tok -> bf16) and issues the y store on the
ACT HWDGE ring; DVE does absmax, round-finish, qT PSUM->SBUF copy, c_tok
and the fused bias add; PE does 4 transposes and 16 bf16 matmuls.
GpSimd only issues the one-time broadcast-bias DMA (Q7 tensor ops
measure ~14 cyc/elem -- never put per-tile tensor work there).
"""

import os
import sys

import numpy as np

B, S, DIN, DOUT = 8, 4096, 512, 2048
N_CORES = 8
KC = DIN // 128  # 4 k-tiles
OC = DOUT // 128  # 16 weight row chunks

RND_A = 1536.0  # fp16 ulp is 1.0 on [1024,2048): fp16(v+1536) rounds v to int
MAGIC = 12582912.0  # 1.5*2^23: fl32(v+MAGIC) == MAGIC + round-half-even(v)
EPS = 1e-6
N_EARLY = 5  # heads emitted before the W stats / quant pipeline

_cached = {}


def _ensure_path():
    try:
        import concourse  # noqa: F401
    except ImportError:
        for p in ("/opt/trn_rl_repo", os.path.expanduser("~/.axon_site/_ro/trn_rl_repo")):
            if os.path.isdir(p) and p not in sys.path:
                sys.path.insert(0, p)


def build_program(s_tiles=S // 128):
    """Emit the Bass/Tile program for one core: x [s_tiles*128, DIN] -> y."""
    _ensure_path()
    from contextlib import ExitStack

    import concourse.bacc as bacc
    import concourse.tile as tile
    from concourse import mybir
    from concourse.masks import make_identity

    f32 = mybir.dt.float32
    f16 = mybir.dt.float16
    bf16 = mybir.dt.bfloat16
    Alu = mybir.AluOpType
    X = mybir.AxisListType.X
    Copy = mybir.ActivationFunctionType.Copy
    SROWS = s_tiles * 128

    nc = bacc.Bacc("TRN2", target_bir_lowering=False, debug=False, num_devices=N_CORES)
    x_d = nc.dram_tensor("x", [SROWS, DIN], f32, kind="ExternalInput").ap()
    w_d = nc.dram_tensor("w", [DOUT, DIN], f32, kind="ExternalInput").ap()
    b_d = nc.dram_tensor("bias", [1, DOUT], f32, kind="ExternalInput").ap()
    y_d = nc.dram_tensor("y", [SROWS, DOUT], bf16, kind="ExternalOutput").ap()

    with tile.TileContext(nc) as tc, ExitStack() as ctx:
        cpool = ctx.enter_context(tc.tile_pool(name="const", bufs=1))
        wallp = ctx.enter_context(tc.tile_pool(name="wall", bufs=1))
        wtmpp = ctx.enter_context(tc.tile_pool(name="wtmp", bufs=4))
        wqp = ctx.enter_context(tc.tile_pool(name="wq", bufs=4))
        statp = ctx.enter_context(tc.tile_pool(name="stat", bufs=1))
        xp = ctx.enter_context(tc.tile_pool(name="x", bufs=8))
        r1p = ctx.enter_context(tc.tile_pool(name="r1", bufs=6))
        qp = ctx.enter_context(tc.tile_pool(name="q", bufs=6))
        qtp = ctx.enter_context(tc.tile_pool(name="qt", bufs=9))
        mxp = ctx.enter_context(tc.tile_pool(name="mx", bufs=48))
        yp = ctx.enter_context(tc.tile_pool(name="y", bufs=6))
        pt = ctx.enter_context(tc.tile_pool(name="ptrans", bufs=2, space="PSUM"))
        py = ctx.enter_context(tc.tile_pool(name="py", bufs=3, space="PSUM"))

        # ---- constants ----
        b_row = cpool.tile([1, DOUT], f32)
        nc.scalar.dma_start(b_row[:], b_d)
        bias_bc = cpool.tile([128, DOUT], bf16)
        nc.gpsimd.dma_start(bias_bc[:], b_d.broadcast_to([128, DOUT]))
        ones128 = cpool.tile([128, 128], f32)
        nc.vector.memset(ones128[:], 1.0)
        ident = cpool.tile([128, 128], bf16)
        make_identity(nc, ident[:])

        # ---- W load: sync ring, 4 consolidated quarter-DMAs (one trigger
        # each -- per-chunk triggers jam the HWDGE ring FIFO) ----
        w_all = wallp.tile([128, OC, DIN], f32)
        w_r = w_d.rearrange("(c p) d -> p c d", p=128)
        for qtr in range(4):
            nc.sync.dma_start(w_all[:, 4 * qtr : 4 * qtr + 4, :], w_r[:, 4 * qtr : 4 * qtr + 4, :])

        # first N_EARLY x tiles in ONE consolidated DMA on the scalar ring
        x_early = cpool.tile([128, N_EARLY, DIN], f32)
        nc.scalar.dma_start(
            x_early[:], x_d[0 : N_EARLY * 128, :].rearrange("(t p) d -> p t d", p=128)
        )

        tT = cpool.tile([128, KC, DOUT], bf16)

        def tile_head(i):
            """x load + activation quant + transpose for token tile i.

            Stats-independent: nothing here waits on mean(|W|)."""
            if i < N_EARLY:
                xt = x_early[:, i, :]
            else:
                xtile = xp.tile([128, DIN], f32)
                nc.sync.dma_start(xtile[:], x_d[i * 128 : (i + 1) * 128, :])
                xt = xtile[:]

            mx = mxp.tile([128, 1], f32, tag="mx")
            nc.vector.tensor_reduce(mx[:], xt, axis=X, op=Alu.max, apply_absolute_value=True)
            sx = mxp.tile([128, 1], f32, tag="sx")
            nc.vector.reciprocal(sx[:], mx[:])
            sx127 = mxp.tile([128, 1], f32, tag="sx127")
            nc.vector.tensor_scalar(sx127[:], sx[:], 127.0, None, op0=Alu.mult)

            # r1 = fp16(x*sx127 + 1536) == 1536 + round(x*sx127), exactly
            r1 = r1p.tile([128, DIN], f16)
            nc.scalar.activation(r1[:], xt, Copy, bias=RND_A, scale=sx127[:])
            q = qp.tile([128, DIN], bf16)
            nc.vector.tensor_scalar(q[:], r1[:], RND_A, None, op0=Alu.subtract)

            pq = pt.tile([128, KC, 128], bf16, tag="tp", name="pq")
            for k in range(KC):
                nc.tensor.transpose(pq[:, k, :], q[:, k * 128 : (k + 1) * 128], ident[:])
            qT = qtp.tile([128, KC, 128], bf16)
            nc.vector.tensor_copy(qT[:], pq[:])
            return qT, mx

        ysbs = {}
        ctoks = {}

        def body_half(i, h, qT, mx):
            """8 bf16 matmuls + epilogue for half h of token tile i;
            bias + store close the tile at h == 1."""
            if h == 0:
                ctoks[i] = mxp.tile([128, 1], f32, tag="ct", name="ct")
                nc.vector.tensor_tensor(ctoks[i][:], mx[:], vw127[:], op=Alu.mult)
                ysbs[i] = yp.tile([128, DOUT], bf16, name="ysb")
            c_tok, ysb = ctoks[i], ysbs[i]
            ph = py.tile([128, 1024], f32, tag="ytile", name="ph")
            base = h * 1024
            for k in range(KC):
                lhsT = qT[:, k, :]
                for n in range(2):
                    nc.tensor.matmul(
                        ph[:, n * 512 : (n + 1) * 512], lhsT,
                        tT[:, k, base + n * 512 : base + (n + 1) * 512],
                        start=(k == 0), stop=(k == KC - 1),
                    )
            nc.scalar.activation(ysb[:, base : base + 1024], ph[:], Copy, scale=c_tok[:])
            if h == 1:
                nc.vector.tensor_tensor(ysb[:], ysb[:], bias_bc[:], op=Alu.add)
                nc.scalar.dma_start(y_d[i * 128 : (i + 1) * 128, :], ysb[:])
                del ysbs[i], ctoks[i]

        def tile_body(i, qT, mx):
            body_half(i, 0, qT, mx)
            body_half(i, 1, qT, mx)

        # ---- early heads: keep DVE/ACT/PE busy while W streams in ----
        heads = {}
        for i in range(N_EARLY):
            heads[i] = tile_head(i)

        # ---- mean(|W|): per-chunk reduces pipeline behind the W DMAs ----
        wsum = statp.tile([128, OC], f32)
        for c in range(OC):
            nc.vector.tensor_reduce(
                wsum[:, c : c + 1], w_all[:, c, :],
                axis=X, op=Alu.add, apply_absolute_value=True,
            )
        tot = statp.tile([128, 1], f32)
        nc.vector.tensor_reduce(tot[:], wsum[:], axis=X, op=Alu.add)
        pred = py.tile([128, 1024], f32, tag="ytile", name="pred")
        nc.tensor.matmul(pred[:, 0:1], ones128[:], tot[:], start=True, stop=True)
        redo = statp.tile([128, 1], f32)
        nc.vector.tensor_copy(redo[:], pred[:, 0:1])
        mean_t = statp.tile([128, 1], f32)
        nc.vector.tensor_scalar(mean_t[:], redo[:], 1.0 / (DOUT * DIN), EPS, op0=Alu.mult, op1=Alu.max)
        s_w = statp.tile([128, 1], f32)  # 1/mean: the weight quantization scale
        nc.vector.reciprocal(s_w[:], mean_t[:])
        v_w = statp.tile([128, 1], f32)  # fl(1/s_w): dequant magnitude (matches ref)
        nc.vector.reciprocal(v_w[:], s_w[:])
        vw127 = statp.tile([128, 1], f32)  # v_w / 127, folded for the epilogue scale
        nc.vector.tensor_scalar(vw127[:], v_w[:], 1.0 / 127.0, None, op0=Alu.mult)

        # ---- W quantize (ternary in bf16) + PE-transpose into tT ----
        def w_quant_group(g):
            wqs = []
            for ci_ in range(4):
                c = g * 4 + ci_
                # Single-rounding fp32 magic: fl32(w*s_w + 1.5*2^23) rounds
                # w*s_w to int directly (a bf16/fp16 output-convert trick
                # would double-round and flip weights that sit ~1e-7 from a
                # ternary boundary -- one flipped weight corrupts a whole
                # output column by v_w*|x|, ~3e-2 relative).
                wr1 = wtmpp.tile([128, DIN], f32, tag="wr1")
                nc.scalar.activation(wr1[:], w_all[:, c, :], Copy, bias=MAGIC, scale=s_w[:])
                wr2 = wtmpp.tile([128, DIN], bf16, tag="wr2")
                nc.vector.tensor_scalar(wr2[:], wr1[:], MAGIC + 1.0, MAGIC, op0=Alu.min, op1=Alu.subtract)
                wq = wqp.tile([128, DIN], bf16)
                nc.vector.tensor_scalar(wq[:], wr2[:], -1.0, None, op0=Alu.max)
                wqs.append(wq)
            for k in range(KC):
                ptk = pt.tile([128, 4, 128], bf16, tag="tp", name="ptk")
                for ci_ in range(4):
                    nc.tensor.transpose(
                        ptk[:, ci_, :], wqs[ci_][:, k * 128 : (k + 1) * 128], ident[:]
                    )
                nc.scalar.copy(tT[:, k, g * 512 : (g + 1) * 512], ptk[:])

        # g0,g1 first: they cover output columns [0,1024) = the h=0 halves,
        # so the first bodies' h0 matmuls can run while g2,g3 still quantize.
        N_SPLIT = 4  # tiles whose h0/h1 halves are emitted around g2,g3
        w_quant_group(0)
        w_quant_group(1)
        heads[N_EARLY] = tile_head(N_EARLY)
        for i in range(N_SPLIT):
            body_half(i, 0, *heads[i])
        w_quant_group(2)
        heads[N_EARLY + 1] = tile_head(N_EARLY + 1)
        w_quant_group(3)
        heads[N_EARLY + 2] = tile_head(N_EARLY + 2)
        for i in range(N_SPLIT):
            body_half(i, 1, *heads.pop(i))

        # ---- main loop (heads roll ~4 tiles ahead of bodies) ----
        for i in range(N_SPLIT, s_tiles):
            j = i + 4
            if j < s_tiles and j not in heads:
                heads[j] = tile_head(j)
            tile_body(i, *heads.pop(i))

    nc.compile()
    return nc


def _get_program():
    if "nc" not in _cached:
        _cached["nc"] = build_program()
    return _cached["nc"]


def kernel(x: np.ndarray, weight: np.ndarray, bias: np.ndarray) -> np.ndarray:
    _ensure_path()
    from concourse.bass_utils import run_bass_kernel_spmd

    x = np.ascontiguousarray(x, dtype=np.float32)
    weight = np.ascontiguousarray(weight, dtype=np.float32)
    bias2d = np.ascontiguousarray(bias, dtype=np.float32).reshape(1, DOUT)

    nc = _get_program()
    in_maps = [{"x": x[c], "w": weight, "bias": bias2d} for c in range(N_CORES)]
    res = run_bass_kernel_spmd(nc, in_maps, core_ids=list(range(N_CORES)))
    _cached["last_results"] = res
    y = np.stack(
        [res.results[c]["y"].astype(np.float32) for c in range(N_CORES)], axis=0
    )
    return y


# revision 23
# speedup vs baseline: 1.0368x; 1.0368x over previous
"""BitLinear (BitNet b1.58) forward kernel for Trainium2, 8 NeuronCores.

y = act_quant(x) @ weight_quant(W)^T + bias
  - activation quant: per-token absmax int8 fake-quant (values in [-127,127])
  - weight quant: per-tensor mean-absmax ternary fake-quant {-1,0,1}

Sharding: data-parallel over the batch dim (8 batches -> 1 per core);
W and bias are replicated per core, each core computes mean(|W|) locally
(no collectives needed).

Design notes (tolerance is rel_err < 2e-2 vs output absmax):
  * The matmul runs in bf16: the PE streams its moving operand at
    2 B/lane/cycle no matter the dtype (fp8 DoubleRow packs 2 k-tiles in
    the same bytes = 2x flops, but exact int8 activations then need hi+lo
    fp8 pairs = the same 2 bytes -> parity with bf16; single-fp8
    activations lose ~2.4% > tolerance). q int8 in bf16 (exact), ternary
    weights in bf16 (exact), fp32 PSUM accumulation (exact).
  * Activation round via output-convert RNE: fp16(x*sx127 + 1536) ==
    1536 + round(x*sx127) exactly (fp16 ulp is 1.0 on [1024,2048)); the
    rare double-round tie flips one token-element by 1 quantum (~2e-4),
    harmless. Weights use the single-rounding fp32 +1.5*2^23 magic --
    a weight flip corrupts a whole output column (~3e-2), not harmless.
  * y is stored to HBM as bf16 (halves store traffic) and upcast to fp32
    on the host after the gather. Bias is one fused 2x-rate DVE add on
    the bf16 tile -- the PE does matmuls and transposes only.
  * Startup: W rides the sync ring (16 chunk DMAs so the per-chunk
    abs-sum reduces pipeline behind them), the first 5 x tiles ride the
    scalar ring so they land while W streams; tile heads are
    stats-independent (c_tok is computed at body time) so nothing in a
    head waits for mean(|W|).

Engine layout per token tile: ACT does the x-scale+round pass and the
two epilogue passes (PSUM * c_tok -> bf16) and issues the y store on the
ACT HWDGE ring; DVE does absmax, round-finish, qT PSUM->SBUF copy, c_tok
and the fused bias add; PE does 4 transposes and 16 bf16 matmuls.
GpSimd only issues the one-time broadcast-bias DMA (Q7 tensor ops
measure ~14 cyc/elem -- never put per-tile tensor work there).
"""

import os
import sys

import numpy as np

B, S, DIN, DOUT = 8, 4096, 512, 2048
N_CORES = 8
KC = DIN // 128  # 4 k-tiles
OC = DOUT // 128  # 16 weight row chunks

RND_A = 1536.0  # fp16 ulp is 1.0 on [1024,2048): fp16(v+1536) rounds v to int
MAGIC = 12582912.0  # 1.5*2^23: fl32(v+MAGIC) == MAGIC + round-half-even(v)
EPS = 1e-6
N_EARLY = 5  # heads emitted before the W stats / quant pipeline

_cached = {}


def _ensure_path():
    try:
        import concourse  # noqa: F401
    except ImportError:
        for p in ("/opt/trn_rl_repo", os.path.expanduser("~/.axon_site/_ro/trn_rl_repo")):
            if os.path.isdir(p) and p not in sys.path:
                sys.path.insert(0, p)


def build_program(s_tiles=S // 128):
    """Emit the Bass/Tile program for one core: x [s_tiles*128, DIN] -> y."""
    _ensure_path()
    from contextlib import ExitStack

    import concourse.bacc as bacc
    import concourse.tile as tile
    from concourse import mybir
    from concourse.masks import make_identity

    f32 = mybir.dt.float32
    f16 = mybir.dt.float16
    bf16 = mybir.dt.bfloat16
    Alu = mybir.AluOpType
    X = mybir.AxisListType.X
    Copy = mybir.ActivationFunctionType.Copy
    SROWS = s_tiles * 128

    nc = bacc.Bacc("TRN2", target_bir_lowering=False, debug=False, num_devices=N_CORES)
    x_d = nc.dram_tensor("x", [SROWS, DIN], f32, kind="ExternalInput").ap()
    w_d = nc.dram_tensor("w", [DOUT, DIN], f32, kind="ExternalInput").ap()
    b_d = nc.dram_tensor("bias", [1, DOUT], f32, kind="ExternalInput").ap()
    y_d = nc.dram_tensor("y", [SROWS, DOUT], bf16, kind="ExternalOutput").ap()

    with tile.TileContext(nc) as tc, ExitStack() as ctx:
        cpool = ctx.enter_context(tc.tile_pool(name="const", bufs=1))
        wallp = ctx.enter_context(tc.tile_pool(name="wall", bufs=1))
        wtmpp = ctx.enter_context(tc.tile_pool(name="wtmp", bufs=4))
        wqp = ctx.enter_context(tc.tile_pool(name="wq", bufs=4))
        statp = ctx.enter_context(tc.tile_pool(name="stat", bufs=1))
        xp = ctx.enter_context(tc.tile_pool(name="x", bufs=8))
        r1p = ctx.enter_context(tc.tile_pool(name="r1", bufs=6))
        qp = ctx.enter_context(tc.tile_pool(name="q", bufs=6))
        qtp = ctx.enter_context(tc.tile_pool(name="qt", bufs=9))
        mxp = ctx.enter_context(tc.tile_pool(name="mx", bufs=48))
        yp = ctx.enter_context(tc.tile_pool(name="y", bufs=4))
        pt = ctx.enter_context(tc.tile_pool(name="ptrans", bufs=2, space="PSUM"))
        py = ctx.enter_context(tc.tile_pool(name="py", bufs=3, space="PSUM"))

        # ---- constants ----
        b_row = cpool.tile([1, DOUT], f32)
        nc.scalar.dma_start(b_row[:], b_d)
        bias_bc = cpool.tile([128, DOUT], bf16)
        nc.gpsimd.dma_start(bias_bc[:], b_d.broadcast_to([128, DOUT]))
        ones128 = cpool.tile([128, 128], f32)
        nc.vector.memset(ones128[:], 1.0)
        ident = cpool.tile([128, 128], bf16)
        make_identity(nc, ident[:])

        # ---- W load: sync ring, ahead of all but the first x tiles ----
        w_all = wallp.tile([128, OC, DIN], f32)
        w_r = w_d.rearrange("(c p) d -> p c d", p=128)
        for c in range(OC):
            nc.sync.dma_start(w_all[:, c : c + 1, :], w_r[:, c : c + 1, :])

        tT = cpool.tile([128, KC, DOUT], bf16)

        def tile_head(i):
            """x load + activation quant + transpose for token tile i.

            Stats-independent: nothing here waits on mean(|W|)."""
            xt = xp.tile([128, DIN], f32)
            eng = nc.scalar if i < N_EARLY else nc.sync
            eng.dma_start(xt[:], x_d[i * 128 : (i + 1) * 128, :])

            mx = mxp.tile([128, 1], f32, tag="mx")
            nc.vector.tensor_reduce(mx[:], xt[:], axis=X, op=Alu.max, apply_absolute_value=True)
            sx = mxp.tile([128, 1], f32, tag="sx")
            nc.vector.reciprocal(sx[:], mx[:])
            sx127 = mxp.tile([128, 1], f32, tag="sx127")
            nc.vector.tensor_scalar(sx127[:], sx[:], 127.0, None, op0=Alu.mult)

            # r1 = fp16(x*sx127 + 1536) == 1536 + round(x*sx127), exactly
            r1 = r1p.tile([128, DIN], f16)
            nc.scalar.activation(r1[:], xt[:], Copy, bias=RND_A, scale=sx127[:])
            q = qp.tile([128, DIN], bf16)
            nc.vector.tensor_scalar(q[:], r1[:], RND_A, None, op0=Alu.subtract)

            pq = pt.tile([128, KC, 128], bf16, tag="tp", name="pq")
            for k in range(KC):
                nc.tensor.transpose(pq[:, k, :], q[:, k * 128 : (k + 1) * 128], ident[:])
            qT = qtp.tile([128, KC, 128], bf16)
            nc.vector.tensor_copy(qT[:], pq[:])
            return qT, mx

        def tile_body(i, qT, mx):
            """16 bf16 matmuls + epilogue + bias + store for token tile i."""
            c_tok = mxp.tile([128, 1], f32, tag="ct")
            nc.vector.tensor_tensor(c_tok[:], mx[:], vw127[:], op=Alu.mult)
            ysb = yp.tile([128, DOUT], bf16)
            for h in range(2):
                ph = py.tile([128, 1024], f32, tag="ytile", name="ph")
                base = h * 1024
                for k in range(KC):
                    lhsT = qT[:, k, :]
                    for n in range(2):
                        nc.tensor.matmul(
                            ph[:, n * 512 : (n + 1) * 512], lhsT,
                            tT[:, k, base + n * 512 : base + (n + 1) * 512],
                            start=(k == 0), stop=(k == KC - 1),
                        )
                nc.scalar.activation(ysb[:, base : base + 1024], ph[:], Copy, scale=c_tok[:])
            nc.vector.tensor_tensor(ysb[:], ysb[:], bias_bc[:], op=Alu.add)
            nc.scalar.dma_start(y_d[i * 128 : (i + 1) * 128, :], ysb[:])

        # ---- early heads: keep DVE/ACT/PE busy while W streams in ----
        heads = {}
        for i in range(N_EARLY):
            heads[i] = tile_head(i)

        # ---- mean(|W|): per-chunk reduces pipeline behind the W DMAs ----
        wsum = statp.tile([128, OC], f32)
        for c in range(OC):
            nc.vector.tensor_reduce(
                wsum[:, c : c + 1], w_all[:, c, :],
                axis=X, op=Alu.add, apply_absolute_value=True,
            )
        tot = statp.tile([128, 1], f32)
        nc.vector.tensor_reduce(tot[:], wsum[:], axis=X, op=Alu.add)
        pred = py.tile([128, 1024], f32, tag="ytile", name="pred")
        nc.tensor.matmul(pred[:, 0:1], ones128[:], tot[:], start=True, stop=True)
        redo = statp.tile([128, 1], f32)
        nc.vector.tensor_copy(redo[:], pred[:, 0:1])
        mean_t = statp.tile([128, 1], f32)
        nc.vector.tensor_scalar(mean_t[:], redo[:], 1.0 / (DOUT * DIN), EPS, op0=Alu.mult, op1=Alu.max)
        s_w = statp.tile([128, 1], f32)  # 1/mean: the weight quantization scale
        nc.vector.reciprocal(s_w[:], mean_t[:])
        v_w = statp.tile([128, 1], f32)  # fl(1/s_w): dequant magnitude (matches ref)
        nc.vector.reciprocal(v_w[:], s_w[:])
        vw127 = statp.tile([128, 1], f32)  # v_w / 127, folded for the epilogue scale
        nc.vector.tensor_scalar(vw127[:], v_w[:], 1.0 / 127.0, None, op0=Alu.mult)

        # ---- W quantize (ternary in bf16) + PE-transpose into tT ----
        def w_quant_group(g):
            wqs = []
            for ci_ in range(4):
                c = g * 4 + ci_
                # Single-rounding fp32 magic: fl32(w*s_w + 1.5*2^23) rounds
                # w*s_w to int directly (a bf16/fp16 output-convert trick
                # would double-round and flip weights that sit ~1e-7 from a
                # ternary boundary -- one flipped weight corrupts a whole
                # output column by v_w*|x|, ~3e-2 relative).
                wr1 = wtmpp.tile([128, DIN], f32, tag="wr1")
                nc.scalar.activation(wr1[:], w_all[:, c, :], Copy, bias=MAGIC, scale=s_w[:])
                wr2 = wtmpp.tile([128, DIN], bf16, tag="wr2")
                nc.vector.tensor_scalar(wr2[:], wr1[:], MAGIC + 1.0, MAGIC, op0=Alu.min, op1=Alu.subtract)
                wq = wqp.tile([128, DIN], bf16)
                nc.vector.tensor_scalar(wq[:], wr2[:], -1.0, None, op0=Alu.max)
                wqs.append(wq)
            for k in range(KC):
                ptk = pt.tile([128, 4, 128], bf16, tag="tp", name="ptk")
                for ci_ in range(4):
                    nc.tensor.transpose(
                        ptk[:, ci_, :], wqs[ci_][:, k * 128 : (k + 1) * 128], ident[:]
                    )
                nc.scalar.copy(tT[:, k, g * 512 : (g + 1) * 512], ptk[:])

        for g in range(OC // 4):
            w_quant_group(g)
            heads[N_EARLY + g] = tile_head(N_EARLY + g)

        # ---- main loop ----
        PREFETCH = N_EARLY + OC // 4  # 9 heads already emitted
        for i in range(s_tiles):
            if i + PREFETCH < s_tiles:
                heads[i + PREFETCH] = tile_head(i + PREFETCH)
            tile_body(i, *heads.pop(i))

    nc.compile()
    return nc


def _get_program():
    if "nc" not in _cached:
        _cached["nc"] = build_program()
    return _cached["nc"]


def kernel(x: np.ndarray, weight: np.ndarray, bias: np.ndarray) -> np.ndarray:
    _ensure_path()
    from concourse.bass_utils import run_bass_kernel_spmd

    x = np.ascontiguousarray(x, dtype=np.float32)
    weight = np.ascontiguousarray(weight, dtype=np.float32)
    bias2d = np.ascontiguousarray(bias, dtype=np.float32).reshape(1, DOUT)

    nc = _get_program()
    in_maps = [{"x": x[c], "w": weight, "bias": bias2d} for c in range(N_CORES)]
    res = run_bass_kernel_spmd(nc, in_maps, core_ids=list(range(N_CORES)))
    _cached["last_results"] = res
    y = np.stack(
        [res.results[c]["y"].astype(np.float32) for c in range(N_CORES)], axis=0
    )
    return y


# revision 25
# speedup vs baseline: 1.0534x; 1.0160x over previous
"""BitLinear (BitNet b1.58) forward kernel for Trainium2, 8 NeuronCores.

y = act_quant(x) @ weight_quant(W)^T + bias
  - activation quant: per-token absmax int8 fake-quant (values in [-127,127])
  - weight quant: per-tensor mean-absmax ternary fake-quant {-1,0,1}

Sharding: data-parallel over the batch dim (8 batches -> 1 per core);
W and bias are replicated per core, each core computes mean(|W|) locally
(no collectives needed).

Design notes (tolerance is rel_err < 2e-2 vs output absmax):
  * The matmul runs in bf16: the PE streams its moving operand at
    2 B/lane/cycle no matter the dtype (fp8 DoubleRow packs 2 k-tiles in
    the same bytes = 2x flops, but exact int8 activations then need hi+lo
    fp8 pairs = the same 2 bytes -> parity with bf16; single-fp8
    activations lose ~2.4% > tolerance). q int8 in bf16 (exact), ternary
    weights in bf16 (exact), fp32 PSUM accumulation (exact).
  * Activation round via output-convert RNE: fp16(x*sx127 + 1536) ==
    1536 + round(x*sx127) exactly (fp16 ulp is 1.0 on [1024,2048)); the
    rare double-round tie flips one token-element by 1 quantum (~2e-4),
    harmless. Weights use the single-rounding fp32 +1.5*2^23 magic --
    a weight flip corrupts a whole output column (~3e-2), not harmless.
  * y is stored to HBM as bf16 (halves store traffic) and upcast to fp32
    on the host after the gather. Bias is one fused 2x-rate DVE add on
    the bf16 tile -- the PE does matmuls and transposes only.
  * Startup: W rides the sync ring (16 chunk DMAs so the per-chunk
    abs-sum reduces pipeline behind them), the first 5 x tiles ride the
    scalar ring so they land while W streams; tile heads are
    stats-independent (c_tok is computed at body time) so nothing in a
    head waits for mean(|W|).

Engine layout per token tile: ACT does the x-scale+round pass and the
two epilogue passes (PSUM * c_tok -> bf16) and issues the y store on the
ACT HWDGE ring; DVE does absmax, round-finish, qT PSUM->SBUF copy, c_tok
and the fused bias add; PE does 4 transposes and 16 bf16 matmuls.
GpSimd only issues the one-time broadcast-bias DMA (Q7 tensor ops
measure ~14 cyc/elem -- never put per-tile tensor work there).
"""

import os
import sys

import numpy as np

B, S, DIN, DOUT = 8, 4096, 512, 2048
N_CORES = 8
KC = DIN // 128  # 4 k-tiles
OC = DOUT // 128  # 16 weight row chunks

RND_A = 1536.0  # fp16 ulp is 1.0 on [1024,2048): fp16(v+1536) rounds v to int
MAGIC = 12582912.0  # 1.5*2^23: fl32(v+MAGIC) == MAGIC + round-half-even(v)
EPS = 1e-6
N_EARLY = 5  # heads emitted before the W stats / quant pipeline

_cached = {}


def _ensure_path():
    try:
        import concourse  # noqa: F401
    except ImportError:
        for p in ("/opt/trn_rl_repo", os.path.expanduser("~/.axon_site/_ro/trn_rl_repo")):
            if os.path.isdir(p) and p not in sys.path:
                sys.path.insert(0, p)


def build_program(s_tiles=S // 128):
    """Emit the Bass/Tile program for one core: x [s_tiles*128, DIN] -> y."""
    _ensure_path()
    from contextlib import ExitStack

    import concourse.bacc as bacc
    import concourse.tile as tile
    from concourse import mybir
    from concourse.masks import make_identity

    f32 = mybir.dt.float32
    f16 = mybir.dt.float16
    bf16 = mybir.dt.bfloat16
    Alu = mybir.AluOpType
    X = mybir.AxisListType.X
    Copy = mybir.ActivationFunctionType.Copy
    SROWS = s_tiles * 128

    nc = bacc.Bacc("TRN2", target_bir_lowering=False, debug=False, num_devices=N_CORES)
    x_d = nc.dram_tensor("x", [SROWS, DIN], f32, kind="ExternalInput").ap()
    w_d = nc.dram_tensor("w", [DOUT, DIN], f32, kind="ExternalInput").ap()
    b_d = nc.dram_tensor("bias", [1, DOUT], f32, kind="ExternalInput").ap()
    y_d = nc.dram_tensor("y", [SROWS, DOUT], bf16, kind="ExternalOutput").ap()

    with tile.TileContext(nc) as tc, ExitStack() as ctx:
        cpool = ctx.enter_context(tc.tile_pool(name="const", bufs=1))
        wallp = ctx.enter_context(tc.tile_pool(name="wall", bufs=1))
        wtmpp = ctx.enter_context(tc.tile_pool(name="wtmp", bufs=4))
        wqp = ctx.enter_context(tc.tile_pool(name="wq", bufs=4))
        statp = ctx.enter_context(tc.tile_pool(name="stat", bufs=1))
        xp = ctx.enter_context(tc.tile_pool(name="x", bufs=8))
        r1p = ctx.enter_context(tc.tile_pool(name="r1", bufs=6))
        qp = ctx.enter_context(tc.tile_pool(name="q", bufs=6))
        qtp = ctx.enter_context(tc.tile_pool(name="qt", bufs=9))
        mxp = ctx.enter_context(tc.tile_pool(name="mx", bufs=48))
        yp = ctx.enter_context(tc.tile_pool(name="y", bufs=4))
        pt = ctx.enter_context(tc.tile_pool(name="ptrans", bufs=2, space="PSUM"))
        py = ctx.enter_context(tc.tile_pool(name="py", bufs=3, space="PSUM"))

        # ---- constants ----
        ones128 = cpool.tile([128, 128], f32)
        nc.vector.memset(ones128[:], 1.0)
        ident = cpool.tile([128, 128], bf16)
        make_identity(nc, ident[:])

        # ---- W load: sync ring, ahead of all but the first x tiles ----
        w_all = wallp.tile([128, OC, DIN], f32)
        w_r = w_d.rearrange("(c p) d -> p c d", p=128)
        for c in range(OC):
            nc.sync.dma_start(w_all[:, c : c + 1, :], w_r[:, c : c + 1, :])

        # bias broadcast after the W DMAs so it doesn't eat early HBM BW
        # (it is first needed ~30us in, at the first body's bias add)
        bias_bc = cpool.tile([128, DOUT], bf16)
        nc.gpsimd.dma_start(bias_bc[:], b_d.broadcast_to([128, DOUT]))

        tT = cpool.tile([128, KC, DOUT], bf16)

        def tile_head(i):
            """x load + activation quant + transpose for token tile i.

            Stats-independent: nothing here waits on mean(|W|)."""
            xt = xp.tile([128, DIN], f32)
            eng = nc.scalar if i < N_EARLY else nc.sync
            eng.dma_start(xt[:], x_d[i * 128 : (i + 1) * 128, :])

            mx = mxp.tile([128, 1], f32, tag="mx")
            nc.vector.tensor_reduce(mx[:], xt[:], axis=X, op=Alu.max, apply_absolute_value=True)
            sx = mxp.tile([128, 1], f32, tag="sx")
            nc.vector.reciprocal(sx[:], mx[:])
            sx127 = mxp.tile([128, 1], f32, tag="sx127")
            nc.vector.tensor_scalar(sx127[:], sx[:], 127.0, None, op0=Alu.mult)

            # r1 = fp16(x*sx127 + 1536) == 1536 + round(x*sx127), exactly
            r1 = r1p.tile([128, DIN], f16)
            nc.scalar.activation(r1[:], xt[:], Copy, bias=RND_A, scale=sx127[:])
            q = qp.tile([128, DIN], bf16)
            nc.vector.tensor_scalar(q[:], r1[:], RND_A, None, op0=Alu.subtract)

            pq = pt.tile([128, KC, 128], bf16, tag="tp", name="pq")
            for k in range(KC):
                nc.tensor.transpose(pq[:, k, :], q[:, k * 128 : (k + 1) * 128], ident[:])
            qT = qtp.tile([128, KC, 128], bf16)
            nc.vector.tensor_copy(qT[:], pq[:])
            return qT, mx

        def tile_body(i, qT, mx):
            """16 bf16 matmuls + epilogue + bias + store for token tile i."""
            c_tok = mxp.tile([128, 1], f32, tag="ct")
            nc.vector.tensor_tensor(c_tok[:], mx[:], vw127[:], op=Alu.mult)
            ysb = yp.tile([128, DOUT], bf16)
            for h in range(2):
                ph = py.tile([128, 1024], f32, tag="ytile", name="ph")
                base = h * 1024
                for k in range(KC):
                    lhsT = qT[:, k, :]
                    for n in range(2):
                        nc.tensor.matmul(
                            ph[:, n * 512 : (n + 1) * 512], lhsT,
                            tT[:, k, base + n * 512 : base + (n + 1) * 512],
                            start=(k == 0), stop=(k == KC - 1),
                        )
                nc.scalar.activation(ysb[:, base : base + 1024], ph[:], Copy, scale=c_tok[:])
            nc.vector.tensor_tensor(ysb[:], ysb[:], bias_bc[:], op=Alu.add)
            nc.scalar.dma_start(y_d[i * 128 : (i + 1) * 128, :], ysb[:])

        # ---- early heads: keep DVE/ACT/PE busy while W streams in ----
        heads = {}
        for i in range(N_EARLY):
            heads[i] = tile_head(i)

        # ---- mean(|W|): per-chunk reduces pipeline behind the W DMAs ----
        wsum = statp.tile([128, OC], f32)
        for c in range(OC):
            nc.vector.tensor_reduce(
                wsum[:, c : c + 1], w_all[:, c, :],
                axis=X, op=Alu.add, apply_absolute_value=True,
            )
        tot = statp.tile([128, 1], f32)
        nc.vector.tensor_reduce(tot[:], wsum[:], axis=X, op=Alu.add)
        pred = py.tile([128, 1024], f32, tag="ytile", name="pred")
        nc.tensor.matmul(pred[:, 0:1], ones128[:], tot[:], start=True, stop=True)
        redo = statp.tile([128, 1], f32)
        nc.vector.tensor_copy(redo[:], pred[:, 0:1])
        mean_t = statp.tile([128, 1], f32)
        nc.vector.tensor_scalar(mean_t[:], redo[:], 1.0 / (DOUT * DIN), EPS, op0=Alu.mult, op1=Alu.max)
        s_w = statp.tile([128, 1], f32)  # 1/mean: the weight quantization scale
        nc.vector.reciprocal(s_w[:], mean_t[:])
        v_w = statp.tile([128, 1], f32)  # fl(1/s_w): dequant magnitude (matches ref)
        nc.vector.reciprocal(v_w[:], s_w[:])
        vw127 = statp.tile([128, 1], f32)  # v_w / 127, folded for the epilogue scale
        nc.vector.tensor_scalar(vw127[:], v_w[:], 1.0 / 127.0, None, op0=Alu.mult)

        # ---- W quantize (ternary in bf16) + PE-transpose into tT ----
        # Two mega-groups of [128,1024] double-chunk passes: group g covers
        # output columns [1024g, 1024g+1024) = exactly the h=g halves of the
        # bodies, so the first bodies' h=0 matmuls unblock after group 0.
        def w_quant_group(g):
            wqs = []
            for ci_ in range(4):
                c2 = g * 4 + ci_  # double-chunk: rows for chunks 2*c2, 2*c2+1
                # Single-rounding fp32 magic: fl32(w*s_w + 1.5*2^23) rounds
                # w*s_w to int directly (a bf16/fp16 output-convert trick
                # would double-round and flip weights that sit ~1e-7 from a
                # ternary boundary -- one flipped weight corrupts a whole
                # output column by v_w*|x|, ~3e-2 relative).
                wr1 = wtmpp.tile([128, 2, DIN], f32, tag="wr1")
                nc.scalar.activation(wr1[:], w_all[:, 2 * c2 : 2 * c2 + 2, :], Copy, bias=MAGIC, scale=s_w[:])
                wr2 = wtmpp.tile([128, 2, DIN], bf16, tag="wr2")
                nc.vector.tensor_scalar(wr2[:], wr1[:], MAGIC + 1.0, MAGIC, op0=Alu.min, op1=Alu.subtract)
                wq = wqp.tile([128, 2, DIN], bf16)
                nc.vector.tensor_scalar(wq[:], wr2[:], -1.0, None, op0=Alu.max)
                wqs.append(wq)
            for k in range(KC):
                ptk = pt.tile([128, 8, 128], bf16, tag="tp", name="ptk")
                for ci_ in range(4):
                    for j in range(2):
                        nc.tensor.transpose(
                            ptk[:, 2 * ci_ + j, :], wqs[ci_][:, j, k * 128 : (k + 1) * 128], ident[:]
                        )
                nc.scalar.copy(tT[:, k, g * 1024 : (g + 1) * 1024], ptk[:])

        w_quant_group(0)
        heads[N_EARLY] = tile_head(N_EARLY)
        heads[N_EARLY + 1] = tile_head(N_EARLY + 1)
        w_quant_group(1)
        heads[N_EARLY + 2] = tile_head(N_EARLY + 2)
        heads[N_EARLY + 3] = tile_head(N_EARLY + 3)

        # ---- main loop ----
        PREFETCH = N_EARLY + 4  # 9 heads already emitted
        for i in range(s_tiles):
            if i + PREFETCH < s_tiles:
                heads[i + PREFETCH] = tile_head(i + PREFETCH)
            tile_body(i, *heads.pop(i))

    nc.compile()
    return nc


def _get_program():
    if "nc" not in _cached:
        _cached["nc"] = build_program()
    return _cached["nc"]


def kernel(x: np.ndarray, weight: np.ndarray, bias: np.ndarray) -> np.ndarray:
    _ensure_path()
    from concourse.bass_utils import run_bass_kernel_spmd

    x = np.ascontiguousarray(x, dtype=np.float32)
    weight = np.ascontiguousarray(weight, dtype=np.float32)
    bias2d = np.ascontiguousarray(bias, dtype=np.float32).reshape(1, DOUT)

    nc = _get_program()
    in_maps = [{"x": x[c], "w": weight, "bias": bias2d} for c in range(N_CORES)]
    res = run_bass_kernel_spmd(nc, in_maps, core_ids=list(range(N_CORES)))
    _cached["last_results"] = res
    y = np.stack(
        [res.results[c]["y"].astype(np.float32) for c in range(N_CORES)], axis=0
    )
    return y
